# revision 4
# baseline (speedup 1.0000x reference)
"""MoE (top-2 of 8 experts) Trainium2 kernel, 8-core expert-pair parallel.

Strategy
--------
The reference output depends only on each token's top-2 experts, so the
device computes the *sparse* FFN: 8192 tokens x 2 = 16384 (token, expert)
slots.

Sharding (the big change vs the slot-parallel baseline): the 8 cores form
4 pairs; pair p owns two experts (paired large-count + small-count so the
per-pair slot count is balanced). Within a pair, core 2p computes hidden
dims H[0:2048] and core 2p+1 computes H[2048:4096] of *both* experts, so
each core's weight working set is (2 experts) x (W1 half + W2 half) =
16.8 MB fp16 -- small enough to stay RESIDENT in SBUF (131 KB/partition).
The baseline streamed all 8 experts' weights (134 MB) through every core;
this layout moves 8x less HBM traffic and turns the kernel into a pure
PE-roofline matmul stream.

Host side (all O(N*D) data movement or O(N*E) router math):
  - f64 router (logits -> top-2 + softmax gates), same selection rule as
    jax.lax.top_k (stable order).
  - per-expert token lists, padded to uniform per-position capacities
    (cap0 = max over pairs of the larger expert's count, cap1 likewise for
    the smaller; SPMD needs compile-time-uniform trip counts).
  - gathers + transposes the selected tokens into a per-pair xselT tensor
    (f16), chunk-major so every device load is one contiguous DMA.
  - after the run: adds the two H-half partial outputs, applies b2 + the
    gate weights, combines each token's two slots, and unshards.

Device program per core (dense FFN pipeline at the PE roofline):
  prologue (per rep): 2 monolithic DMAs park W1half/W2half in SBUF.
  for each 512-token chunk j of each local expert:
    L1: for ht in 16: psum <- sum_dt W1[ht,dt].T @ xs[dt]   (8 N=512 MMs)
        hT[ht] = gelu(psum + b1)                            (ACT engine)
    L2: for dt in 8:  psum <- sum_ht W2[dt,ht].T @ hT[ht]   (16 N=512 MMs)
        ot[dt] = psum                                       (DVE copy)
    DMA ot -> DRAM (fp16 H-half partial; host adds the two halves).
  L1(j+1) is emitted before L2(j) (software pipelining) so the PE never
  waits on the ACT drain at a chunk boundary.
"""

import os
import sys

for _p in ("/root/.axon_site/_ro/trn_rl_repo", "/opt/trn_rl_repo"):
    if os.path.isdir(_p) and _p not in sys.path:
        sys.path.insert(0, _p)

import numpy as np

import concourse.bass as bass  # noqa: F401  (kept for parity with utils)
import concourse.bacc as bacc
import concourse.tile as tile
from concourse import mybir
from concourse.bass_utils import run_bass_kernel_spmd

F32 = mybir.dt.float32
F16 = mybir.dt.float16
AF = mybir.ActivationFunctionType

D = 1024      # in_features
H = 4096      # hidden
E = 8         # experts
TOPK = 2
N_CORES = 8
N = 8192      # total tokens
ND = D // 128    # 8 feature tiles
NPAIR = 4
HHALF = H // 2   # 2048 hidden dims per core
NHH = HHALF // 128  # 16 hidden tiles per core
CH = 512         # token chunk (one PSUM bank of fp32)

REPS = 1   # device-side repeat loop (timing only; >1 wraps body in For_i)


# --------------------------------------------------------------------------
# host routing + expert pairing
# --------------------------------------------------------------------------

def _route(x, Wg, bg):
    """f64 router: top-2 expert ids (stable tie-break, like lax.top_k) and
    softmax gate weights."""
    xt = np.asarray(x, np.float64).reshape(-1, D)
    logits = xt @ np.asarray(Wg, np.float64) + np.asarray(bg, np.float64)
    top2 = np.argsort(-logits, axis=1, kind="stable")[:, :TOPK]
    m = logits.max(axis=1, keepdims=True)
    p = np.exp(logits - m)
    p /= p.sum(axis=1, keepdims=True)
    gates = np.take_along_axis(p, top2, axis=1)
    return top2, gates.astype(np.float32)


def _chunk_sizes(cap):
    full, rem = divmod(cap, CH)
    return [CH] * full + ([rem] if rem else [])


class _Plan:
    __slots__ = ("top2", "gates", "pairs", "caps", "SC", "chunks",
                 "slot_tok", "colof", "_pin")


_PLAN_CACHE: dict = {}


def _plan(x, Wg, bg):
    key = (id(x), id(Wg))
    hit = _PLAN_CACHE.get(key)
    if hit is not None:
        return hit
    top2, gates = _route(x, Wg, bg)
    cnt = np.bincount(top2.ravel(), minlength=E)
    # pair the 4 busiest experts (local slot 0) with the 4 least busy
    # (local slot 1): per-position capacity = max count at that position,
    # rounded to a multiple of 8 (16B-aligned f16 slices).
    order = np.argsort(-cnt, kind="stable")
    big4, small4 = order[:NPAIR], order[NPAIR:]
    pairs = [(int(big4[i]), int(small4[i])) for i in range(NPAIR)]
    cap0 = -(-int(cnt[big4].max()) // 8) * 8
    cap1 = -(-int(cnt[small4].max()) // 8) * 8
    caps = (cap0, cap1)
    SC = cap0 + cap1
    # chunk list: (local_expert, slot_offset, ncols)
    chunks = []
    for le, cap in enumerate(caps):
        off = le * cap0
        for c in _chunk_sizes(cap):
            chunks.append((le, off, c))
            off += c
    # slot -> token map per pair + (token, k) -> global column map
    slot_tok = np.zeros((NPAIR, SC), np.int64)
    colof = np.zeros((N, TOPK), np.int64)
    for p, (ea, eb) in enumerate(pairs):
        for le, e in enumerate((ea, eb)):
            off = le * cap0
            tl = np.where((top2 == e).any(axis=1))[0]
            k = len(tl)
            slot_tok[p, off:off + k] = tl
            r = np.where(top2[tl, 0] == e, 0, 1)
            colof[tl, r] = p * SC + off + np.arange(k)
    pl = _Plan()
    pl.top2, pl.gates, pl.pairs, pl.caps, pl.SC = top2, gates, pairs, caps, SC
    pl.chunks, pl.slot_tok, pl.colof = chunks, slot_tok, colof
    pl._pin = (x, Wg)   # keep ids alive for the cache key
    _PLAN_CACHE.clear()
    _PLAN_CACHE[key] = pl
    return pl


def route_capacities(inputs):
    """Compile-time shapes: (cap0, cap1) local-expert capacities."""
    return _plan(inputs["x"], inputs["Wg"], inputs["bg"]).caps


# --------------------------------------------------------------------------
# device program
# --------------------------------------------------------------------------

def _emit_body(nc, tc, io, caps, pools):
    """One repetition of the kernel: weight DMA + all chunk compute.

    Pools and the resident-weight tiles live OUTSIDE the REPS timing loop
    (pool setup/teardown inside a For_i costs a per-iteration drain);
    everything here is the honest per-call work.
    """
    cap0, cap1 = caps
    chunks = []
    for le, cap in enumerate(caps):
        off = le * cap0
        for c in _chunk_sizes(cap):
            chunks.append((le, off, c))
            off += c
    nch = len(chunks)

    if True:
        xpool, hpool, opool, ps1, ps2 = pools["loop"]
        b1c, W1r, W2r = pools["res"]

        nc.scalar.dma_start(b1c[:], io["b1c"].ap())
        # resident weights: one monolithic DMA per layer (8.4 MB each).
        # W1r col = le*NHH*1024 + ht*1024 + dt*128 + j   (j = H dim in tile)
        # W2r col = le*ND*2048  + dt*2048 + ht*128 + m   (m = D dim in tile)
        nc.sync.dma_start(W1r[:], io["W1c"].ap())
        nc.sync.dma_start(W2r[:], io["W2s"].ap())

        Xd = io["xseltc"].ap()     # [128, ND*SC] f16, chunk-major blocks
        Od = io["out"].ap()        # [128, ND*SC] f16, chunk-major blocks

        def load_xs(j):
            _, off, C = chunks[j]
            xs = xpool.tile([128, ND * C], F16, tag="xs", name=f"xs{j}")
            nc.gpsimd.dma_start(xs[:], Xd[:, ND * off: ND * off + ND * C])
            return xs

        def emit_l1(j, xs):
            le, _, C = chunks[j]
            hT = hpool.tile([128, NHH * C], F16, tag="hT", name=f"hT{j}")
            for ht in range(NHH):
                w1off = (le * NHH + ht) * 1024
                psf = ps1.tile([128, 512], F32, tag="ps1", name="psf1")
                ps = psf[:, :C]
                for dt in range(ND):
                    nc.tensor.matmul(
                        ps[:],
                        lhsT=W1r[:, w1off + dt * 128: w1off + (dt + 1) * 128],
                        rhs=xs[:, dt * C:(dt + 1) * C],
                        start=(dt == 0),
                        stop=(dt == ND - 1),
                    )
                bidx = le * NHH + ht
                nc.scalar.activation(
                    hT[:, ht * C:(ht + 1) * C],
                    ps[:],
                    AF.Gelu,
                    bias=b1c[:, bidx: bidx + 1],
                )
            return hT

        def emit_l2(j, hT):
            le, off, C = chunks[j]
            ot = opool.tile([128, ND * C], F16, tag="ot", name=f"ot{j}")
            for dt in range(ND):
                w2off = (le * ND + dt) * 2048
                psf = ps2.tile([128, 512], F32, tag="ps2", name="psf2")
                ps = psf[:, :C]
                for ht in range(NHH):
                    nc.tensor.matmul(
                        ps[:],
                        lhsT=W2r[:, w2off + ht * 128: w2off + (ht + 1) * 128],
                        rhs=hT[:, ht * C:(ht + 1) * C],
                        start=(ht == 0),
                        stop=(ht == NHH - 1),
                    )
                nc.vector.tensor_copy(ot[:, dt * C:(dt + 1) * C], ps[:])
            nc.scalar.dma_start(Od[:, ND * off: ND * off + ND * C], ot[:])

        # software-pipelined emission: L1(0), L1(1), L2(0), L1(2), L2(1), ...
        # so the PE runs chunk j+1's layer 1 while ACT drains chunk j's gelu.
        xs_next = load_xs(0)
        hT_prev = None
        for j in range(nch):
            xs = xs_next
            if j + 1 < nch:
                xs_next = load_xs(j + 1)
            hT = emit_l1(j, xs)
            if hT_prev is not None:
                emit_l2(j - 1, hT_prev)
            hT_prev = hT
        emit_l2(nch - 1, hT_prev)


def _build_sparse(caps):
    from contextlib import ExitStack

    nc = bacc.Bacc(None, target_bir_lowering=False, debug=False,
                   num_devices=N_CORES)
    SC = int(sum(caps))
    io = {
        "xseltc": nc.declare_dram_parameter("xseltc", [128, ND * SC], F16,
                                            isOutput=False),
        "W1c": nc.declare_dram_parameter("W1c", [128, 2 * NHH * 1024], F16,
                                         isOutput=False),
        "W2s": nc.declare_dram_parameter("W2s", [128, 2 * ND * 2048], F16,
                                         isOutput=False),
        "b1c": nc.declare_dram_parameter("b1c", [128, 2 * NHH], F32,
                                         isOutput=False),
        "out": nc.declare_dram_parameter("out", [128, ND * SC], F16,
                                         isOutput=True),
    }
    with tile.TileContext(nc) as tc, ExitStack() as ctx:
        cpool = ctx.enter_context(tc.tile_pool(name="const", bufs=1))
        wpool = ctx.enter_context(tc.tile_pool(name="wres", bufs=1))
        xpool = ctx.enter_context(tc.tile_pool(name="xs", bufs=2))
        hpool = ctx.enter_context(tc.tile_pool(name="hT", bufs=2))
        opool = ctx.enter_context(tc.tile_pool(name="ot", bufs=2))
        ps1 = ctx.enter_context(tc.tile_pool(name="ps1", bufs=5, space="PSUM"))
        ps2 = ctx.enter_context(tc.tile_pool(name="ps2", bufs=3, space="PSUM"))
        b1c = cpool.tile([128, 2 * NHH], F32)
        W1r = wpool.tile([128, 2 * NHH * 1024], F16)
        W2r = wpool.tile([128, 2 * ND * 2048], F16)
        pools = {
            "loop": (xpool, hpool, opool, ps1, ps2),
            "res": (b1c, W1r, W2r),
        }
        if REPS > 1:
            with tc.For_i(0, REPS, 1):
                _emit_body(nc, tc, io, caps, pools)
        else:
            _emit_body(nc, tc, io, caps, pools)
    nc.compile()
    return nc


# --------------------------------------------------------------------------
# host prep / combine
# --------------------------------------------------------------------------

def prep_inputs(x, Wg, bg, W1, b1, W2, b2):
    """Host-side shard + layout/dtype prep. Returns per-core input maps."""
    plan = _plan(x, Wg, bg)
    cap0, cap1 = plan.caps
    SC = plan.SC

    xt = np.asarray(x, np.float32).reshape(-1, D).astype(np.float16)
    W1h = np.asarray(W1, np.float32).astype(np.float16)          # [E, D, H]
    W2h = np.asarray(W2, np.float32).astype(np.float16)          # [E, H, D]
    b1f = np.asarray(b1, np.float32)                             # [E, H]

    in_maps = []
    for p, (ea, eb) in enumerate(plan.pairs):
        # per-pair token gather, chunk-major layout
        xseltc = np.empty((128, ND * SC), np.float16)
        for le, off, C in plan.chunks:
            cols = plan.slot_tok[p, off:off + C]
            xsel = xt[cols]                                      # [C, D]
            blk = xsel.T.reshape(ND, 128, C).transpose(1, 0, 2)
            xseltc[:, ND * off: ND * off + ND * C] = blk.reshape(128, ND * C)
        for half in range(2):
            hs = slice(half * HHALF, (half + 1) * HHALF)
            # W1c[p, ht*1024 + dt*128 + j] = W1[e, dt*128+p, half*2048+ht*128+j]
            w1blk = []
            w2blk = []
            b1blk = []
            for e in (ea, eb):
                w1 = W1h[e][:, hs].reshape(ND, 128, NHH, 128)
                w1blk.append(np.ascontiguousarray(
                    w1.transpose(1, 2, 0, 3)).reshape(128, NHH * ND * 128))
                w2 = W2h[e][hs, :].reshape(NHH, 128, ND, 128)
                w2blk.append(np.ascontiguousarray(
                    w2.transpose(1, 2, 0, 3)).reshape(128, ND * NHH * 128))
                b1blk.append(np.ascontiguousarray(
                    b1f[e][hs].reshape(NHH, 128).T))
            in_maps.append({
                "xseltc": xseltc,
                "W1c": np.concatenate(w1blk, axis=1),
                "W2s": np.concatenate(w2blk, axis=1),
                "b1c": np.concatenate(b1blk, axis=1),
            })
    return in_maps


_CACHE = {}


def kernel(x, Wg, bg, W1, b1, W2, b2):
    B_, S_, D_ = x.shape
    plan = _plan(x, Wg, bg)
    SC = plan.SC
    in_maps = prep_inputs(x, Wg, bg, W1, b1, W2, b2)

    key = ("v3", plan.caps)
    if key not in _CACHE:
        _CACHE[key] = _build_sparse(plan.caps)
    nc = _CACHE[key]
    res = run_bass_kernel_spmd(nc, in_maps, list(range(N_CORES)))

    # decode per-pair outputs (sum of the two H-half partials) into
    # eo_all[d, global_slot]
    eo_all = np.empty((D, NPAIR * SC), np.float32)
    for p in range(NPAIR):
        osum = (np.asarray(res.results[2 * p]["out"]).astype(np.float32) +
                np.asarray(res.results[2 * p + 1]["out"]).astype(np.float32))
        for le, off, C in plan.chunks:
            blk = osum[:, ND * off: ND * off + ND * C]
            eo_all[:, p * SC + off: p * SC + off + C] = (
                blk.reshape(128, ND, C).transpose(1, 0, 2).reshape(D, C))

    # combine: out[t] = sum_k g_k * (eo[:, col_k] + b2[e_k])
    b2f = np.asarray(b2, np.float32)
    g = plan.gates
    t2 = plan.top2
    out = g[:, 0:1] * (eo_all[:, plan.colof[:, 0]].T + b2f[t2[:, 0]])
    out += g[:, 1:2] * (eo_all[:, plan.colof[:, 1]].T + b2f[t2[:, 1]])
    return out.reshape(B_, S_, D_).astype(np.float32)


if __name__ == "__main__":
    sys.path.insert(0, "/root/problem")
    npz = "/root/problem/_inputs.npz"
    if os.path.exists(npz):
        dat = np.load(npz)
        inputs = {k: dat[k] for k in ("x", "Wg", "bg", "W1", "b1", "W2", "b2")}
        want = dat["ref"]
    else:
        os.environ.setdefault("JAX_PLATFORMS", "")
        import reference as R
        inputs = {k: np.asarray(v) for k, v in R.setup_inputs().items()}
        want = None

    got = kernel(**inputs)
    if want is not None:
        diff = np.abs(got - want)
        scale = np.abs(want).max()
        rel_fro = np.linalg.norm(diff) / np.linalg.norm(want)
        print(f"absmax err: {diff.max():.3e}  scale: {scale:.3e}  "
              f"absmax/scale: {diff.max() / scale:.3e}  rel_fro: {rel_fro:.3e}")


# revision 28
# speedup vs baseline: 1.1223x; 1.1223x over previous
"""MoE (top-2 of 8 experts) Trainium2 kernel, 8-core expert-pair parallel.

Strategy
--------
The reference output depends only on each token's top-2 experts, so the
device computes the *sparse* FFN: 8192 tokens x 2 = 16384 (token, expert)
slots.

Sharding (the big change vs the slot-parallel baseline): the 8 cores form
4 pairs; pair p owns two experts (paired large-count + small-count so the
per-pair slot count is balanced). Within a pair, core 2p computes hidden
dims H[0:2048] and core 2p+1 computes H[2048:4096] of *both* experts, so
each core's weight working set is (2 experts) x (W1 half + W2 half) =
16.8 MB fp16 -- small enough to stay RESIDENT in SBUF (131 KB/partition).
The baseline streamed all 8 experts' weights (134 MB) through every core;
this layout moves 8x less HBM traffic and turns the kernel into a pure
PE-roofline matmul stream.

Host side (all O(N*D) data movement or O(N*E) router math):
  - f64 router (logits -> top-2 + softmax gates), same selection rule as
    jax.lax.top_k (stable order).
  - per-expert token lists, padded to uniform per-position capacities
    (cap0 = max over pairs of the larger expert's count, cap1 likewise for
    the smaller; SPMD needs compile-time-uniform trip counts).
  - gathers + transposes the selected tokens into a per-pair xselT tensor
    (f16), chunk-major so every device load is one contiguous DMA.
  - after the run: adds the two H-half partial outputs, applies b2 + the
    gate weights, combines each token's two slots, and unshards.

Device program per core (dense FFN pipeline at the PE roofline):
  prologue (per rep): 2 monolithic DMAs park W1half/W2half in SBUF.
  for each 512-token chunk j of each local expert:
    L1: for ht in 16: psum <- sum_dt W1[ht,dt].T @ xs[dt]   (8 N=512 MMs)
        hT[ht] = gelu(psum + b1)                            (ACT engine)
    L2: for dt in 8:  psum <- sum_ht W2[dt,ht].T @ hT[ht]   (16 N=512 MMs)
        ot[dt] = psum                                       (DVE copy)
    DMA ot -> DRAM (fp16 H-half partial; host adds the two halves).
  L1(j+1) is emitted before L2(j) (software pipelining) so the PE never
  waits on the ACT drain at a chunk boundary.
"""

import os
import sys

for _p in ("/root/.axon_site/_ro/trn_rl_repo", "/opt/trn_rl_repo"):
    if os.path.isdir(_p) and _p not in sys.path:
        sys.path.insert(0, _p)

import numpy as np

import concourse.bass as bass  # noqa: F401  (kept for parity with utils)
import concourse.bacc as bacc
import concourse.tile as tile
from concourse import mybir
from concourse.bass_utils import run_bass_kernel_spmd

F32 = mybir.dt.float32
F16 = mybir.dt.float16
AF = mybir.ActivationFunctionType

D = 1024      # in_features
H = 4096      # hidden
E = 8         # experts
TOPK = 2
N_CORES = 8
N = 8192      # total tokens
ND = D // 128    # 8 feature tiles
NPAIR = 4
HHALF = H // 2   # 2048 hidden dims per core
NHH = HHALF // 128  # 16 hidden tiles per core
CH = 512         # token chunk (one PSUM bank of fp32)
XS_BUFS = 2      # xs landing buffers (DMA side)

REPS = 1   # device-side repeat loop (timing only; >1 wraps body in For_i)
BODY_DUP = 1   # bodies emitted per For_i iteration (timing diagnostics)
SKIP = set()   # timing ablations: subsets of {"xdma","wdma","odma","act","copy"}
XDMA_REAL = None   # ablation: set of chunk ids whose xs is really consumed


# --------------------------------------------------------------------------
# host routing + expert pairing
# --------------------------------------------------------------------------

def _route(x, Wg, bg):
    """f64 router: top-2 expert ids (stable tie-break, like lax.top_k) and
    softmax gate weights."""
    xt = np.asarray(x, np.float64).reshape(-1, D)
    logits = xt @ np.asarray(Wg, np.float64) + np.asarray(bg, np.float64)
    top2 = np.argsort(-logits, axis=1, kind="stable")[:, :TOPK]
    m = logits.max(axis=1, keepdims=True)
    p = np.exp(logits - m)
    p /= p.sum(axis=1, keepdims=True)
    gates = np.take_along_axis(p, top2, axis=1)
    return top2, gates.astype(np.float32)


def _chunk_sizes(cap):
    full, rem = divmod(cap, CH)
    return [CH] * full + ([rem] if rem else [])


class _Plan:
    __slots__ = ("top2", "gates", "pairs", "caps", "SC", "chunks",
                 "slot_tok", "colof", "_pin")


_PLAN_CACHE: dict = {}


def _plan(x, Wg, bg):
    key = (id(x), id(Wg))
    hit = _PLAN_CACHE.get(key)
    if hit is not None:
        return hit
    top2, gates = _route(x, Wg, bg)
    cnt = np.bincount(top2.ravel(), minlength=E)
    # pair the 4 busiest experts (local slot 0) with the 4 least busy
    # (local slot 1): per-position capacity = max count at that position,
    # rounded to a multiple of 8 (16B-aligned f16 slices).
    order = np.argsort(-cnt, kind="stable")
    big4, small4 = order[:NPAIR], order[NPAIR:]
    pairs = [(int(big4[i]), int(small4[i])) for i in range(NPAIR)]
    cap0 = -(-int(cnt[big4].max()) // 8) * 8
    cap1 = -(-int(cnt[small4].max()) // 8) * 8
    caps = (cap0, cap1)
    SC = cap0 + cap1
    # chunk list: (local_expert, slot_offset, ncols)
    chunks = []
    for le, cap in enumerate(caps):
        off = le * cap0
        for c in _chunk_sizes(cap):
            chunks.append((le, off, c))
            off += c
    # slot -> token map per pair + (token, k) -> global column map
    slot_tok = np.zeros((NPAIR, SC), np.int64)
    colof = np.zeros((N, TOPK), np.int64)
    for p, (ea, eb) in enumerate(pairs):
        for le, e in enumerate((ea, eb)):
            off = le * cap0
            tl = np.where((top2 == e).any(axis=1))[0]
            k = len(tl)
            slot_tok[p, off:off + k] = tl
            r = np.where(top2[tl, 0] == e, 0, 1)
            colof[tl, r] = p * SC + off + np.arange(k)
    pl = _Plan()
    pl.top2, pl.gates, pl.pairs, pl.caps, pl.SC = top2, gates, pairs, caps, SC
    pl.chunks, pl.slot_tok, pl.colof = chunks, slot_tok, colof
    pl._pin = (x, Wg)   # keep ids alive for the cache key
    _PLAN_CACHE.clear()
    _PLAN_CACHE[key] = pl
    return pl


def route_capacities(inputs):
    """Compile-time shapes: (cap0, cap1) local-expert capacities."""
    return _plan(inputs["x"], inputs["Wg"], inputs["bg"]).caps


# --------------------------------------------------------------------------
# device program
# --------------------------------------------------------------------------

def _emit_body(nc, tc, io, caps, pools):
    """One repetition of the kernel: weight DMA + all chunk compute.

    Pools and the resident-weight tiles live OUTSIDE the REPS timing loop
    (pool setup/teardown inside a For_i costs a per-iteration drain);
    everything here is the honest per-call work.
    """
    cap0, cap1 = caps
    chunks = []
    for le, cap in enumerate(caps):
        off = le * cap0
        for c in _chunk_sizes(cap):
            chunks.append((le, off, c))
            off += c
    nch = len(chunks)

    if True:
        xpool, hpool, opool, ps1, ps2, xcpool = pools["loop"]
        b1c, W1r, W2r = pools["res"]
        stat = pools["static"]

        XDMA = "xdma" not in SKIP
        XPHANTOM = "xphantom" in SKIP
        WDMA = "wdma" not in SKIP
        ODMA = "odma" not in SKIP
        ACT = "act" not in SKIP
        COPY = "copy" not in SKIP

        Xd = io["xseltc"].ap()     # [128, ND*SC] f16, chunk-major blocks
        Od = io["out"].ap()        # [128, ND*SC] f16, chunk-major blocks

        # The For_i loop boundary is a cross-engine barrier, so NO transfer
        # of iteration i+1 starts before iteration i's consumers are done --
        # cross-iteration prefetch is impossible. The whole DMA train
        # therefore rides ONE HWDGE ring (SP, otherwise idle) in strict
        # first-use order: xs(0) first, then W1 pieces in consumption order
        # with the remaining xs chunks interleaved, then W2 pieces. The PE
        # starts ~10 us into the iteration and the train stays ahead of it.
        # W1r col = le*NHH*1024 + ht*1024 + dt*128 + j   (j = H dim in tile)
        # W2r col = le*ND*2048  + dt*2048 + ht*128 + m   (m = D dim in tile)
        xs_tiles = {}
        xgroups = [tuple(g for g in (2 * k, 2 * k + 1) if g < nch)
                   for k in range((nch + 1) // 2)]

        def load_xs_group(g):
            if not XDMA:
                return
            off = chunks[g[0]][1]
            tot = sum(chunks[j][2] for j in g)
            xs = xpool.tile([128, ND * tot], F16, tag="xs", name=f"xsg{g[0]}")
            nc.gpsimd.dma_start(xs[:], Xd[:, ND * off: ND * off + ND * tot])
            if XPHANTOM:
                return
            base = 0
            for j in g:
                C = chunks[j][2]
                if XDMA_REAL is None or j in XDMA_REAL:
                    xs_tiles[j] = xs[:, base: base + ND * C]
                base += ND * C

        w1w, w2w = NHH * 1024, ND * 2048   # cols per local expert

        def load_w1(le, k, n=4):           # piece k of n for local expert le
            if not WDMA:
                return
            s = slice(le * w1w + k * w1w // n, le * w1w + (k + 1) * w1w // n)
            nc.sync.dma_start(W1r[:, s], io["W1c"].ap()[:, s])

        def load_w2(le, k, n=4):
            if not WDMA:
                return
            s = slice(le * w2w + k * w2w // n, le * w2w + (k + 1) * w2w // n)
            nc.sync.dma_start(W2r[:, s], io["W2s"].ap()[:, s])

        # two parallel DMA trains: weights (16.8 MB) on the SP/HWDGE ring
        # in consumption order; xs chunks (1 MB each) via SWDGE, whose
        # triggers all fire from the idle Pool queue at body start. The
        # SDMA engines round-robin the rings, so the early xs chunks are
        # not stuck behind the weight reload.
        if ACT:
            nc.sync.dma_start(b1c[:], io["b1c"].ap())
        for le in range(2):
            for k in range(4):
                load_w1(le, k)
            for k in range(4):
                load_w2(le, k)
        for g in xgroups:
            load_xs_group(g)

        def emit_l1(j, xs):
            le, _, C = chunks[j]
            if xs is None:
                xs = stat["xs"][:, :ND * C]
            hT = (hpool.tile([128, NHH * C], F16, tag="hT", name=f"hT{j}")
                  if ACT else None)
            for ht in range(NHH):
                w1off = (le * NHH + ht) * 1024
                psf = ps1.tile([128, 512], F32, tag="ps1", name="psf1")
                ps = psf[:, :C]
                for dt in range(ND):
                    nc.tensor.matmul(
                        ps[:],
                        lhsT=W1r[:, w1off + dt * 128: w1off + (dt + 1) * 128],
                        rhs=xs[:, dt * C:(dt + 1) * C],
                        start=(dt == 0),
                        stop=(dt == ND - 1),
                    )
                if ACT:
                    bidx = le * NHH + ht
                    nc.scalar.activation(
                        hT[:, ht * C:(ht + 1) * C],
                        ps[:],
                        AF.Gelu,
                        bias=b1c[:, bidx: bidx + 1],
                    )
            return hT

        def emit_l2(j, hT):
            le, off, C = chunks[j]
            if hT is None:
                hT = stat["hT"][:, :NHH * C]
            HD = ND // 2
            ots = ([opool.tile([128, HD * C], F16, tag="ot", name=f"ot{j}{h}")
                    for h in range(2)]
                   if COPY else [stat["ot"][:, :HD * C]] * 2)
            for dt in range(ND):
                w2off = (le * ND + dt) * 2048
                psf = ps2.tile([128, 512], F32, tag="ps2", name="psf2")
                ps = psf[:, :C]
                for ht in range(NHH):
                    nc.tensor.matmul(
                        ps[:],
                        lhsT=W2r[:, w2off + ht * 128: w2off + (ht + 1) * 128],
                        rhs=hT[:, ht * C:(ht + 1) * C],
                        start=(ht == 0),
                        stop=(ht == NHH - 1),
                    )
                if COPY:
                    ot = ots[dt // HD]
                    dd = dt % HD
                    nc.vector.tensor_copy(ot[:, dd * C:(dd + 1) * C], ps[:])
                if ODMA and COPY and dt % HD == HD - 1:
                    nc.gpsimd.dma_start(
                        Od[:, ND * off + (dt - HD + 1) * C:
                           ND * off + (dt + 1) * C],
                        ots[dt // HD][:])

        # software-pipelined emission: L1(0), L1(1), L2(0), L1(2), L2(1), ...
        # so the PE runs chunk j+1's layer 1 while ACT drains chunk j's gelu.
        hT_prev = None
        for j in range(nch):
            hT = emit_l1(j, xs_tiles.get(j))
            if hT_prev is not None:
                emit_l2(j - 1, hT_prev)
            hT_prev = hT
        emit_l2(nch - 1, hT_prev)


def _build_sparse(caps):
    from contextlib import ExitStack

    nc = bacc.Bacc(None, target_bir_lowering=False, debug=False,
                   num_devices=N_CORES)
    SC = int(sum(caps))
    io = {
        "xseltc": nc.declare_dram_parameter("xseltc", [128, ND * SC], F16,
                                            isOutput=False),
        "W1c": nc.declare_dram_parameter("W1c", [128, 2 * NHH * 1024], F16,
                                         isOutput=False),
        "W2s": nc.declare_dram_parameter("W2s", [128, 2 * ND * 2048], F16,
                                         isOutput=False),
        "b1c": nc.declare_dram_parameter("b1c", [128, 2 * NHH], F32,
                                         isOutput=False),
        "out": nc.declare_dram_parameter("out", [128, ND * SC], F16,
                                         isOutput=True),
    }
    with tile.TileContext(nc) as tc, ExitStack() as ctx:
        xpool = ctx.enter_context(tc.tile_pool(name="xs", bufs=XS_BUFS))
        cpool = ctx.enter_context(tc.tile_pool(name="const", bufs=1))
        wpool = ctx.enter_context(tc.tile_pool(name="wres", bufs=1))
        xcpool = ctx.enter_context(tc.tile_pool(name="xsc", bufs=2))
        hpool = ctx.enter_context(tc.tile_pool(name="hT", bufs=2))
        opool = ctx.enter_context(tc.tile_pool(name="ot", bufs=3))
        ps1 = ctx.enter_context(tc.tile_pool(name="ps1", bufs=5, space="PSUM"))
        ps2 = ctx.enter_context(tc.tile_pool(name="ps2", bufs=3, space="PSUM"))
        b1c = cpool.tile([128, 2 * NHH], F32)
        W1r = wpool.tile([128, 2 * NHH * 1024], F16)
        W2r = wpool.tile([128, 2 * ND * 2048], F16)
        pools = {
            "loop": (xpool, hpool, opool, ps1, ps2, xcpool),
            "res": (b1c, W1r, W2r),
            "static": {},
        }
        # static stand-in tiles for ablation builds (written once, pre-loop)
        if "wdma" in SKIP:
            nc.sync.dma_start(W1r[:], io["W1c"].ap())
            nc.sync.dma_start(W2r[:], io["W2s"].ap())
        if "xdma" in SKIP or "xphantom" in SKIP or XDMA_REAL is not None:
            xst = cpool.tile([128, ND * CH], F16)
            nc.vector.memset(xst[:], 0.25)
            pools["static"]["xs"] = xst
        if "act" in SKIP:
            hst = cpool.tile([128, NHH * CH], F16)
            nc.vector.memset(hst[:], 0.25)
            pools["static"]["hT"] = hst
        if "copy" in SKIP:
            ost = cpool.tile([128, ND * CH], F16)
            nc.vector.memset(ost[:], 0.25)
            pools["static"]["ot"] = ost
        if REPS > 1:
            with tc.For_i(0, REPS, 1):
                for _ in range(BODY_DUP):
                    _emit_body(nc, tc, io, caps, pools)
        else:
            _emit_body(nc, tc, io, caps, pools)
    nc.compile()
    return nc


# --------------------------------------------------------------------------
# host prep / combine
# --------------------------------------------------------------------------

def prep_inputs(x, Wg, bg, W1, b1, W2, b2):
    """Host-side shard + layout/dtype prep. Returns per-core input maps."""
    plan = _plan(x, Wg, bg)
    cap0, cap1 = plan.caps
    SC = plan.SC

    xt = np.asarray(x, np.float32).reshape(-1, D).astype(np.float16)
    W1h = np.asarray(W1, np.float32).astype(np.float16)          # [E, D, H]
    W2h = np.asarray(W2, np.float32).astype(np.float16)          # [E, H, D]
    b1f = np.asarray(b1, np.float32)                             # [E, H]

    in_maps = []
    for p, (ea, eb) in enumerate(plan.pairs):
        # per-pair token gather, chunk-major layout
        xseltc = np.empty((128, ND * SC), np.float16)
        for le, off, C in plan.chunks:
            cols = plan.slot_tok[p, off:off + C]
            xsel = xt[cols]                                      # [C, D]
            blk = xsel.T.reshape(ND, 128, C).transpose(1, 0, 2)
            xseltc[:, ND * off: ND * off + ND * C] = blk.reshape(128, ND * C)
        for half in range(2):
            hs = slice(half * HHALF, (half + 1) * HHALF)
            # W1c[p, ht*1024 + dt*128 + j] = W1[e, dt*128+p, half*2048+ht*128+j]
            w1blk = []
            w2blk = []
            b1blk = []
            for e in (ea, eb):
                w1 = W1h[e][:, hs].reshape(ND, 128, NHH, 128)
                w1blk.append(np.ascontiguousarray(
                    w1.transpose(1, 2, 0, 3)).reshape(128, NHH * ND * 128))
                w2 = W2h[e][hs, :].reshape(NHH, 128, ND, 128)
                w2blk.append(np.ascontiguousarray(
                    w2.transpose(1, 2, 0, 3)).reshape(128, ND * NHH * 128))
                b1blk.append(np.ascontiguousarray(
                    b1f[e][hs].reshape(NHH, 128).T))
            in_maps.append({
                "xseltc": xseltc,
                "W1c": np.concatenate(w1blk, axis=1),
                "W2s": np.concatenate(w2blk, axis=1),
                "b1c": np.concatenate(b1blk, axis=1),
            })
    return in_maps


_CACHE = {}


def kernel(x, Wg, bg, W1, b1, W2, b2):
    B_, S_, D_ = x.shape
    plan = _plan(x, Wg, bg)
    SC = plan.SC
    in_maps = prep_inputs(x, Wg, bg, W1, b1, W2, b2)

    key = ("v3", plan.caps)
    if key not in _CACHE:
        _CACHE[key] = _build_sparse(plan.caps)
    nc = _CACHE[key]
    res = run_bass_kernel_spmd(nc, in_maps, list(range(N_CORES)))

    # decode per-pair outputs (sum of the two H-half partials) into
    # eo_all[d, global_slot]
    eo_all = np.empty((D, NPAIR * SC), np.float32)
    for p in range(NPAIR):
        osum = (np.asarray(res.results[2 * p]["out"]).astype(np.float32) +
                np.asarray(res.results[2 * p + 1]["out"]).astype(np.float32))
        for le, off, C in plan.chunks:
            blk = osum[:, ND * off: ND * off + ND * C]
            eo_all[:, p * SC + off: p * SC + off + C] = (
                blk.reshape(128, ND, C).transpose(1, 0, 2).reshape(D, C))

    # combine: out[t] = sum_k g_k * (eo[:, col_k] + b2[e_k])
    b2f = np.asarray(b2, np.float32)
    g = plan.gates
    t2 = plan.top2
    out = g[:, 0:1] * (eo_all[:, plan.colof[:, 0]].T + b2f[t2[:, 0]])
    out += g[:, 1:2] * (eo_all[:, plan.colof[:, 1]].T + b2f[t2[:, 1]])
    return out.reshape(B_, S_, D_).astype(np.float32)


if __name__ == "__main__":
    sys.path.insert(0, "/root/problem")
    npz = "/root/problem/_inputs.npz"
    if os.path.exists(npz):
        dat = np.load(npz)
        inputs = {k: dat[k] for k in ("x", "Wg", "bg", "W1", "b1", "W2", "b2")}
        want = dat["ref"]
    else:
        os.environ.setdefault("JAX_PLATFORMS", "")
        import reference as R
        inputs = {k: np.asarray(v) for k, v in R.setup_inputs().items()}
        want = None

    got = kernel(**inputs)
    if want is not None:
        diff = np.abs(got - want)
        scale = np.abs(want).max()
        rel_fro = np.linalg.norm(diff) / np.linalg.norm(want)
        print(f"absmax err: {diff.max():.3e}  scale: {scale:.3e}  "
              f"absmax/scale: {diff.max() / scale:.3e}  rel_fro: {rel_fro:.3e}")


# revision 29
# speedup vs baseline: 1.1237x; 1.0012x over previous
"""MoE (top-2 of 8 experts) Trainium2 kernel, 8-core expert-pair parallel.

Strategy
--------
The reference output depends only on each token's top-2 experts, so the
device computes the *sparse* FFN: 8192 tokens x 2 = 16384 (token, expert)
slots.

Sharding (the big change vs the slot-parallel baseline): the 8 cores form
4 pairs; pair p owns two experts (paired large-count + small-count so the
per-pair slot count is balanced). Within a pair, core 2p computes hidden
dims H[0:2048] and core 2p+1 computes H[2048:4096] of *both* experts, so
each core's weight working set is (2 experts) x (W1 half + W2 half) =
16.8 MB fp16 -- small enough to stay RESIDENT in SBUF (131 KB/partition).
The baseline streamed all 8 experts' weights (134 MB) through every core;
this layout moves 8x less HBM traffic and turns the kernel into a pure
PE-roofline matmul stream.

Host side (all O(N*D) data movement or O(N*E) router math):
  - f64 router (logits -> top-2 + softmax gates), same selection rule as
    jax.lax.top_k (stable order).
  - per-expert token lists, padded to uniform per-position capacities
    (cap0 = max over pairs of the larger expert's count, cap1 likewise for
    the smaller; SPMD needs compile-time-uniform trip counts).
  - gathers + transposes the selected tokens into a per-pair xselT tensor
    (f16), chunk-major so every device load is one contiguous DMA.
  - after the run: adds the two H-half partial outputs, applies b2 + the
    gate weights, combines each token's two slots, and unshards.

Device program per core (dense FFN pipeline at the PE roofline):
  prologue (per rep): weight pieces stream into resident SBUF tiles on the
  SP/HWDGE ring in consumption order; xs chunk loads ride SWDGE (gpsimd)
  whose triggers all fire from the idle Pool queue at body start; out
  stores ride SWDGE behind them. Keeping DMA triggers off the ACT ring
  matters: a trigger's lane-reuse wait blocks the issuing engine's queue,
  and ACT must keep draining gelus.
  for each 512-token chunk j of each local expert:
    L1: for ht in 16: psum <- sum_dt W1[ht,dt].T @ xs[dt]   (8 N=512 MMs)
        hT[ht] = gelu(psum + b1)                            (ACT engine)
    L2: for dt in 8:  psum <- sum_ht W2[dt,ht].T @ hT[ht]   (16 N=512 MMs)
        ot[dt] = psum                                       (DVE copy)
    DMA ot -> DRAM (fp16 H-half partial; host adds the two halves).
  L1(j+1) is emitted before L2(j) (software pipelining) so the PE never
  waits on the ACT drain at a chunk boundary.
"""

import os
import sys

for _p in ("/root/.axon_site/_ro/trn_rl_repo", "/opt/trn_rl_repo"):
    if os.path.isdir(_p) and _p not in sys.path:
        sys.path.insert(0, _p)

import numpy as np

import concourse.bass as bass  # noqa: F401  (kept for parity with utils)
import concourse.bacc as bacc
import concourse.tile as tile
from concourse import mybir
from concourse.bass_utils import run_bass_kernel_spmd

F32 = mybir.dt.float32
F16 = mybir.dt.float16
AF = mybir.ActivationFunctionType

D = 1024      # in_features
H = 4096      # hidden
E = 8         # experts
TOPK = 2
N_CORES = 8
N = 8192      # total tokens
ND = D // 128    # 8 feature tiles
NPAIR = 4
HHALF = H // 2   # 2048 hidden dims per core
NHH = HHALF // 128  # 16 hidden tiles per core
CH = 512         # token chunk (one PSUM bank of fp32)
XS_BUFS = 2      # xs landing buffers (DMA side)

REPS = 1   # device-side repeat loop (timing only; >1 wraps body in For_i)
BODY_DUP = 1   # bodies emitted per For_i iteration (timing diagnostics)
SKIP = set()   # timing ablations: subsets of {"xdma","wdma","odma","act","copy"}
XDMA_REAL = None   # ablation: set of chunk ids whose xs is really consumed


# --------------------------------------------------------------------------
# host routing + expert pairing
# --------------------------------------------------------------------------

def _route(x, Wg, bg):
    """f64 router: top-2 expert ids (stable tie-break, like lax.top_k) and
    softmax gate weights."""
    xt = np.asarray(x, np.float64).reshape(-1, D)
    logits = xt @ np.asarray(Wg, np.float64) + np.asarray(bg, np.float64)
    top2 = np.argsort(-logits, axis=1, kind="stable")[:, :TOPK]
    m = logits.max(axis=1, keepdims=True)
    p = np.exp(logits - m)
    p /= p.sum(axis=1, keepdims=True)
    gates = np.take_along_axis(p, top2, axis=1)
    return top2, gates.astype(np.float32)


def _chunk_sizes(cap):
    full, rem = divmod(cap, CH)
    return [CH] * full + ([rem] if rem else [])


class _Plan:
    __slots__ = ("top2", "gates", "pairs", "caps", "SC", "chunks",
                 "slot_tok", "colof", "_pin")


_PLAN_CACHE: dict = {}


def _plan(x, Wg, bg):
    key = (id(x), id(Wg))
    hit = _PLAN_CACHE.get(key)
    if hit is not None:
        return hit
    top2, gates = _route(x, Wg, bg)
    cnt = np.bincount(top2.ravel(), minlength=E)
    # pair the 4 busiest experts (local slot 0) with the 4 least busy
    # (local slot 1): per-position capacity = max count at that position,
    # rounded to a multiple of 8 (16B-aligned f16 slices).
    order = np.argsort(-cnt, kind="stable")
    big4, small4 = order[:NPAIR], order[NPAIR:]
    pairs = [(int(big4[i]), int(small4[i])) for i in range(NPAIR)]
    cap0 = -(-int(cnt[big4].max()) // 8) * 8
    cap1 = -(-int(cnt[small4].max()) // 8) * 8
    caps = (cap0, cap1)
    SC = cap0 + cap1
    # chunk list: (local_expert, slot_offset, ncols)
    chunks = []
    for le, cap in enumerate(caps):
        off = le * cap0
        for c in _chunk_sizes(cap):
            chunks.append((le, off, c))
            off += c
    # slot -> token map per pair + (token, k) -> global column map
    slot_tok = np.zeros((NPAIR, SC), np.int64)
    colof = np.zeros((N, TOPK), np.int64)
    for p, (ea, eb) in enumerate(pairs):
        for le, e in enumerate((ea, eb)):
            off = le * cap0
            tl = np.where((top2 == e).any(axis=1))[0]
            k = len(tl)
            slot_tok[p, off:off + k] = tl
            r = np.where(top2[tl, 0] == e, 0, 1)
            colof[tl, r] = p * SC + off + np.arange(k)
    pl = _Plan()
    pl.top2, pl.gates, pl.pairs, pl.caps, pl.SC = top2, gates, pairs, caps, SC
    pl.chunks, pl.slot_tok, pl.colof = chunks, slot_tok, colof
    pl._pin = (x, Wg)   # keep ids alive for the cache key
    _PLAN_CACHE.clear()
    _PLAN_CACHE[key] = pl
    return pl


def route_capacities(inputs):
    """Compile-time shapes: (cap0, cap1) local-expert capacities."""
    return _plan(inputs["x"], inputs["Wg"], inputs["bg"]).caps


# --------------------------------------------------------------------------
# device program
# --------------------------------------------------------------------------

def _emit_body(nc, tc, io, caps, pools):
    """One repetition of the kernel: weight DMA + all chunk compute.

    Pools and the resident-weight tiles live OUTSIDE the REPS timing loop
    (pool setup/teardown inside a For_i costs a per-iteration drain);
    everything here is the honest per-call work.
    """
    cap0, cap1 = caps
    chunks = []
    for le, cap in enumerate(caps):
        off = le * cap0
        for c in _chunk_sizes(cap):
            chunks.append((le, off, c))
            off += c
    nch = len(chunks)

    if True:
        xpool, hpool, opool, ps1, ps2, xcpool = pools["loop"]
        b1c, W1r, W2r = pools["res"]
        stat = pools["static"]

        XDMA = "xdma" not in SKIP
        XPHANTOM = "xphantom" in SKIP
        WDMA = "wdma" not in SKIP
        ODMA = "odma" not in SKIP
        ACT = "act" not in SKIP
        COPY = "copy" not in SKIP

        Xd = io["xseltc"].ap()     # [128, ND*SC] f16, chunk-major blocks
        Od = io["out"].ap()        # [128, ND*SC] f16, chunk-major blocks

        # The For_i loop boundary is a cross-engine barrier, so NO transfer
        # of iteration i+1 starts before iteration i's consumers are done --
        # cross-iteration prefetch is impossible. The whole DMA train
        # therefore rides ONE HWDGE ring (SP, otherwise idle) in strict
        # first-use order: xs(0) first, then W1 pieces in consumption order
        # with the remaining xs chunks interleaved, then W2 pieces. The PE
        # starts ~10 us into the iteration and the train stays ahead of it.
        # W1r col = le*NHH*1024 + ht*1024 + dt*128 + j   (j = H dim in tile)
        # W2r col = le*ND*2048  + dt*2048 + ht*128 + m   (m = D dim in tile)
        xs_tiles = {}
        xgroups = [tuple(g for g in (2 * k, 2 * k + 1) if g < nch)
                   for k in range((nch + 1) // 2)]

        def load_xs_group(g):
            if not XDMA:
                return
            off = chunks[g[0]][1]
            tot = sum(chunks[j][2] for j in g)
            xs = xpool.tile([128, ND * tot], F16, tag="xs", name=f"xsg{g[0]}")
            nc.gpsimd.dma_start(xs[:], Xd[:, ND * off: ND * off + ND * tot])
            if XPHANTOM:
                return
            base = 0
            for j in g:
                C = chunks[j][2]
                if XDMA_REAL is None or j in XDMA_REAL:
                    xs_tiles[j] = xs[:, base: base + ND * C]
                base += ND * C

        w1w, w2w = NHH * 1024, ND * 2048   # cols per local expert

        def load_w1(le, k, n=4):           # piece k of n for local expert le
            if not WDMA:
                return
            s = slice(le * w1w + k * w1w // n, le * w1w + (k + 1) * w1w // n)
            nc.sync.dma_start(W1r[:, s], io["W1c"].ap()[:, s])

        def load_w2(le, k, n=4):
            if not WDMA:
                return
            s = slice(le * w2w + k * w2w // n, le * w2w + (k + 1) * w2w // n)
            nc.sync.dma_start(W2r[:, s], io["W2s"].ap()[:, s])

        # two parallel DMA trains: weights (16.8 MB) on the SP/HWDGE ring
        # in consumption order; xs chunks (1 MB each) via SWDGE, whose
        # triggers all fire from the idle Pool queue at body start. The
        # SDMA engines round-robin the rings, so the early xs chunks are
        # not stuck behind the weight reload.
        if ACT:
            nc.sync.dma_start(b1c[:], io["b1c"].ap())
        for le in range(2):
            for k in range(4):
                load_w1(le, k)
            for k in range(4):
                load_w2(le, k)
        for g in xgroups:
            load_xs_group(g)

        def emit_l1(j, xs):
            le, _, C = chunks[j]
            if xs is None:
                xs = stat["xs"][:, :ND * C]
            hT = (hpool.tile([128, NHH * C], F16, tag="hT", name=f"hT{j}")
                  if ACT else None)
            for ht in range(NHH):
                w1off = (le * NHH + ht) * 1024
                psf = ps1.tile([128, 512], F32, tag="ps1", name="psf1")
                ps = psf[:, :C]
                for dt in range(ND):
                    nc.tensor.matmul(
                        ps[:],
                        lhsT=W1r[:, w1off + dt * 128: w1off + (dt + 1) * 128],
                        rhs=xs[:, dt * C:(dt + 1) * C],
                        start=(dt == 0),
                        stop=(dt == ND - 1),
                    )
                if ACT:
                    bidx = le * NHH + ht
                    nc.scalar.activation(
                        hT[:, ht * C:(ht + 1) * C],
                        ps[:],
                        AF.Gelu,
                        bias=b1c[:, bidx: bidx + 1],
                    )
            return hT

        def emit_l2(j, hT):
            le, off, C = chunks[j]
            if hT is None:
                hT = stat["hT"][:, :NHH * C]
            HD = ND // 2
            ots = ([opool.tile([128, HD * C], F16, tag="ot", name=f"ot{j}{h}")
                    for h in range(2)]
                   if COPY else [stat["ot"][:, :HD * C]] * 2)
            for dt in range(ND):
                w2off = (le * ND + dt) * 2048
                psf = ps2.tile([128, 512], F32, tag="ps2", name="psf2")
                ps = psf[:, :C]
                for ht in range(NHH):
                    nc.tensor.matmul(
                        ps[:],
                        lhsT=W2r[:, w2off + ht * 128: w2off + (ht + 1) * 128],
                        rhs=hT[:, ht * C:(ht + 1) * C],
                        start=(ht == 0),
                        stop=(ht == NHH - 1),
                    )
                if COPY:
                    ot = ots[dt // HD]
                    dd = dt % HD
                    nc.vector.tensor_copy(ot[:, dd * C:(dd + 1) * C], ps[:])
                if ODMA and COPY and dt % HD == HD - 1:
                    nc.gpsimd.dma_start(
                        Od[:, ND * off + (dt - HD + 1) * C:
                           ND * off + (dt + 1) * C],
                        ots[dt // HD][:])

        # software-pipelined emission: L1(0), L1(1), L2(0), L1(2), L2(1), ...
        # so the PE runs chunk j+1's layer 1 while ACT drains chunk j's gelu.
        hT_prev = None
        for j in range(nch):
            hT = emit_l1(j, xs_tiles.get(j))
            if hT_prev is not None:
                emit_l2(j - 1, hT_prev)
            hT_prev = hT
        emit_l2(nch - 1, hT_prev)


def _build_sparse(caps):
    from contextlib import ExitStack

    nc = bacc.Bacc(None, target_bir_lowering=False, debug=False,
                   num_devices=N_CORES)
    SC = int(sum(caps))
    io = {
        "xseltc": nc.declare_dram_parameter("xseltc", [128, ND * SC], F16,
                                            isOutput=False),
        "W1c": nc.declare_dram_parameter("W1c", [128, 2 * NHH * 1024], F16,
                                         isOutput=False),
        "W2s": nc.declare_dram_parameter("W2s", [128, 2 * ND * 2048], F16,
                                         isOutput=False),
        "b1c": nc.declare_dram_parameter("b1c", [128, 2 * NHH], F32,
                                         isOutput=False),
        "out": nc.declare_dram_parameter("out", [128, ND * SC], F16,
                                         isOutput=True),
    }
    with tile.TileContext(nc) as tc, ExitStack() as ctx:
        xpool = ctx.enter_context(tc.tile_pool(name="xs", bufs=XS_BUFS))
        cpool = ctx.enter_context(tc.tile_pool(name="const", bufs=1))
        wpool = ctx.enter_context(tc.tile_pool(name="wres", bufs=1))
        xcpool = ctx.enter_context(tc.tile_pool(name="xsc", bufs=2))
        hpool = ctx.enter_context(tc.tile_pool(name="hT", bufs=2))
        opool = ctx.enter_context(tc.tile_pool(name="ot", bufs=3))
        ps1 = ctx.enter_context(tc.tile_pool(name="ps1", bufs=5, space="PSUM"))
        ps2 = ctx.enter_context(tc.tile_pool(name="ps2", bufs=3, space="PSUM"))
        b1c = cpool.tile([128, 2 * NHH], F32)
        W1r = wpool.tile([128, 2 * NHH * 1024], F16)
        W2r = wpool.tile([128, 2 * ND * 2048], F16)
        pools = {
            "loop": (xpool, hpool, opool, ps1, ps2, xcpool),
            "res": (b1c, W1r, W2r),
            "static": {},
        }
        # static stand-in tiles for ablation builds (written once, pre-loop)
        if "wdma" in SKIP:
            nc.sync.dma_start(W1r[:], io["W1c"].ap())
            nc.sync.dma_start(W2r[:], io["W2s"].ap())
        if "xdma" in SKIP or "xphantom" in SKIP or XDMA_REAL is not None:
            xst = cpool.tile([128, ND * CH], F16)
            nc.vector.memset(xst[:], 0.25)
            pools["static"]["xs"] = xst
        if "act" in SKIP:
            hst = cpool.tile([128, NHH * CH], F16)
            nc.vector.memset(hst[:], 0.25)
            pools["static"]["hT"] = hst
        if "copy" in SKIP:
            ost = cpool.tile([128, ND * CH], F16)
            nc.vector.memset(ost[:], 0.25)
            pools["static"]["ot"] = ost
        if REPS > 1:
            with tc.For_i(0, REPS, 1):
                for _ in range(BODY_DUP):
                    _emit_body(nc, tc, io, caps, pools)
        else:
            _emit_body(nc, tc, io, caps, pools)
    nc.compile()
    return nc


# --------------------------------------------------------------------------
# host prep / combine
# --------------------------------------------------------------------------

def prep_inputs(x, Wg, bg, W1, b1, W2, b2):
    """Host-side shard + layout/dtype prep. Returns per-core input maps."""
    plan = _plan(x, Wg, bg)
    cap0, cap1 = plan.caps
    SC = plan.SC

    xt = np.asarray(x, np.float32).reshape(-1, D).astype(np.float16)
    W1h = np.asarray(W1, np.float32).astype(np.float16)          # [E, D, H]
    W2h = np.asarray(W2, np.float32).astype(np.float16)          # [E, H, D]
    b1f = np.asarray(b1, np.float32)                             # [E, H]

    in_maps = []
    for p, (ea, eb) in enumerate(plan.pairs):
        # per-pair token gather, chunk-major layout
        xseltc = np.empty((128, ND * SC), np.float16)
        for le, off, C in plan.chunks:
            cols = plan.slot_tok[p, off:off + C]
            xsel = xt[cols]                                      # [C, D]
            blk = xsel.T.reshape(ND, 128, C).transpose(1, 0, 2)
            xseltc[:, ND * off: ND * off + ND * C] = blk.reshape(128, ND * C)
        for half in range(2):
            hs = slice(half * HHALF, (half + 1) * HHALF)
            # W1c[p, ht*1024 + dt*128 + j] = W1[e, dt*128+p, half*2048+ht*128+j]
            w1blk = []
            w2blk = []
            b1blk = []
            for e in (ea, eb):
                w1 = W1h[e][:, hs].reshape(ND, 128, NHH, 128)
                w1blk.append(np.ascontiguousarray(
                    w1.transpose(1, 2, 0, 3)).reshape(128, NHH * ND * 128))
                w2 = W2h[e][hs, :].reshape(NHH, 128, ND, 128)
                w2blk.append(np.ascontiguousarray(
                    w2.transpose(1, 2, 0, 3)).reshape(128, ND * NHH * 128))
                b1blk.append(np.ascontiguousarray(
                    b1f[e][hs].reshape(NHH, 128).T))
            in_maps.append({
                "xseltc": xseltc,
                "W1c": np.concatenate(w1blk, axis=1),
                "W2s": np.concatenate(w2blk, axis=1),
                "b1c": np.concatenate(b1blk, axis=1),
            })
    return in_maps


_CACHE = {}


def kernel(x, Wg, bg, W1, b1, W2, b2):
    B_, S_, D_ = x.shape
    plan = _plan(x, Wg, bg)
    SC = plan.SC
    in_maps = prep_inputs(x, Wg, bg, W1, b1, W2, b2)

    key = ("v3", plan.caps)
    if key not in _CACHE:
        _CACHE[key] = _build_sparse(plan.caps)
    nc = _CACHE[key]
    res = run_bass_kernel_spmd(nc, in_maps, list(range(N_CORES)))

    # decode per-pair outputs (sum of the two H-half partials) into
    # eo_all[d, global_slot]
    eo_all = np.empty((D, NPAIR * SC), np.float32)
    for p in range(NPAIR):
        osum = (np.asarray(res.results[2 * p]["out"]).astype(np.float32) +
                np.asarray(res.results[2 * p + 1]["out"]).astype(np.float32))
        for le, off, C in plan.chunks:
            blk = osum[:, ND * off: ND * off + ND * C]
            eo_all[:, p * SC + off: p * SC + off + C] = (
                blk.reshape(128, ND, C).transpose(1, 0, 2).reshape(D, C))

    # combine: out[t] = sum_k g_k * (eo[:, col_k] + b2[e_k])
    b2f = np.asarray(b2, np.float32)
    g = plan.gates
    t2 = plan.top2
    out = g[:, 0:1] * (eo_all[:, plan.colof[:, 0]].T + b2f[t2[:, 0]])
    out += g[:, 1:2] * (eo_all[:, plan.colof[:, 1]].T + b2f[t2[:, 1]])
    return out.reshape(B_, S_, D_).astype(np.float32)


if __name__ == "__main__":
    sys.path.insert(0, "/root/problem")
    npz = "/root/problem/_inputs.npz"
    if os.path.exists(npz):
        dat = np.load(npz)
        inputs = {k: dat[k] for k in ("x", "Wg", "bg", "W1", "b1", "W2", "b2")}
        want = dat["ref"]
    else:
        os.environ.setdefault("JAX_PLATFORMS", "")
        import reference as R
        inputs = {k: np.asarray(v) for k, v in R.setup_inputs().items()}
        want = None

    got = kernel(**inputs)
    if want is not None:
        diff = np.abs(got - want)
        scale = np.abs(want).max()
        rel_fro = np.linalg.norm(diff) / np.linalg.norm(want)
        print(f"absmax err: {diff.max():.3e}  scale: {scale:.3e}  "
              f"absmax/scale: {diff.max() / scale:.3e}  rel_fro: {rel_fro:.3e}")
